# revision 19
# baseline (speedup 1.0000x reference)
"""Bass/Trainium2 kernel for nn_CharLevelLanguageModel (6-layer char transformer).

v2 strategy: data-parallel over batch (64 -> 8 cores x 8). Per core, each layer
is emitted stage-major across the 4 batch-pair (512-token) tiles so that every
serial chain (LN row ops, softmax normalize) is covered by independent matmul
work from the other batch-pairs.

Key choices vs v1:
- bf16 matmul operands (weights + activations); residual stream and stats stay
  fp32 (f32r) in SBUF; PSUM accumulates fp32.
- Zero GpSimd instructions. Partition broadcasts are ones-column matmuls on the
  PE; elementwise work is split between DVE and ACT.
- Single ACT table set ("natural_log_exp_and_others"): rsqrt(v)=exp(-0.5*ln v),
  1/s = exp(-ln s). No table reloads after startup.
- LN affine params folded into adjacent weights on host; biases ride K=1
  bf16 matmul rows (brows x onesrow) or ACT bias columns.
- Causal mask is one multiplicative bf16 DVE op per (head, batch) unit.
"""

import os
import numpy as np
import ml_dtypes

import concourse.bass as bass
import concourse.mybir as mybir
import concourse.tile as tile
from concourse import bacc
from concourse.bass_utils import run_bass_kernel_spmd

B, T, C, H, L, V = 64, 256, 384, 6, 6, 65
HS = C // H          # 64
DFF = 4 * C          # 1536
N_CORES = 8
BPC = B // N_CORES   # 8 batches per core
NTOK = BPC * T       # 2048 tokens per core
NBP = 4              # batch-pair (512-token) tiles per core
KC = C // 128        # 3 feature chunks
K12 = DFF // 128     # 12 dff chunks
EPS = 1e-5
SCALE = HS ** -0.5

f32 = mybir.dt.float32
f32r = mybir.dt.float32r
bf16 = mybir.dt.bfloat16
i32 = mybir.dt.int32
AF = mybir.ActivationFunctionType
ALU = mybir.AluOpType

N_LAYERS = int(os.environ.get("KERNEL_LAYERS", str(L)))

_cache = {}


def _build_nc():
    nc = bacc.Bacc("TRN2", target_bir_lowering=False, debug=False,
                   num_devices=N_CORES)

    x0T_d = nc.dram_tensor("x0T", [C, NTOK], f32r, kind="ExternalInput").ap()
    wqkv_d = nc.dram_tensor("wqkv", [L, C, 3 * C], bf16, kind="ExternalInput").ap()
    bqkv_d = nc.dram_tensor("bqkv", [L, 128, 6], f32, kind="ExternalInput").ap()
    wo_d = nc.dram_tensor("wo", [L, C, C], bf16, kind="ExternalInput").ap()
    w1_d = nc.dram_tensor("w1", [L, C, DFF], bf16, kind="ExternalInput").ap()
    b1_d = nc.dram_tensor("b1", [L, 128, K12], f32, kind="ExternalInput").ap()
    w2_d = nc.dram_tensor("w2", [L, DFF, C], bf16, kind="ExternalInput").ap()
    brows_d = nc.dram_tensor("brows", [L, 1, 2 * C], bf16, kind="ExternalInput").ap()
    wlm_d = nc.dram_tensor("wlm", [C, V], bf16, kind="ExternalInput").ap()
    blm_d = nc.dram_tensor("blm", [V], f32, kind="ExternalInput").ap()
    m01_d = nc.dram_tensor("m01", [128, 512], bf16, kind="ExternalInput").ap()
    outT_d = nc.dram_tensor("outT", [V, NTOK], f32, kind="ExternalOutput").ap()

    with tile.TileContext(nc) as tc:
        _build_body(nc, tc, x0T_d, wqkv_d, bqkv_d, wo_d, w1_d, b1_d, w2_d,
                    brows_d, wlm_d, blm_d, m01_d, outT_d)
    nc.compile()
    return nc


def _build_body(nc, tc, x0T_d, wqkv_d, bqkv_d, wo_d, w1_d, b1_d, w2_d,
                brows_d, wlm_d, blm_d, m01_d, outT_d):
    import contextlib
    ctx = contextlib.ExitStack()
    p_const = ctx.enter_context(tc.tile_pool(name="consts", bufs=1))
    p_x = ctx.enter_context(tc.tile_pool(name="x", bufs=1))
    p_w = ctx.enter_context(tc.tile_pool(name="w", bufs=1))
    p_big = ctx.enter_context(tc.tile_pool(name="bigsb", bufs=1))   # xsq/xc
    p_xn = ctx.enter_context(tc.tile_pool(name="xn", bufs=1))
    p_rows = ctx.enter_context(tc.tile_pool(name="rows", bufs=1))
    p_qk = ctx.enter_context(tc.tile_pool(name="qk", bufs=1))
    p_v = ctx.enter_context(tc.tile_pool(name="v", bufs=1))
    p_e = ctx.enter_context(tc.tile_pool(name="e", bufs=1))
    p_attc = ctx.enter_context(tc.tile_pool(name="attc", bufs=1))
    p_a = ctx.enter_context(tc.tile_pool(name="a", bufs=1))
    p_out = ctx.enter_context(tc.tile_pool(name="out", bufs=2))
    ps_aux = ctx.enter_context(tc.tile_pool(name="ps_aux", bufs=2, space="PSUM"))
    ps_ap = ctx.enter_context(tc.tile_pool(name="ps_ap", bufs=2, space="PSUM"))
    ps_big = ctx.enter_context(tc.tile_pool(name="ps_big", bufs=4, space="PSUM"))

    # ---- constants ----
    stage = p_const.tile([128, 8], f32, tag="stage")
    onesC = p_const.tile([128, 2], f32r, tag="onesC")      # 1/C for mean matmuls
    nc.vector.memset(stage[:, 0:2], 1.0 / C)
    nc.vector.tensor_copy(onesC[:], stage[:, 0:2])
    stage_row = p_const.tile([1, 512], f32, tag="stage_row")
    nc.vector.memset(stage_row[:, 0:128], 1.0)
    onescol = p_const.tile([1, 128], f32r, tag="onescol")  # bcast lhsT
    nc.vector.tensor_copy(onescol[:], stage_row[:, 0:128])
    onesrow = p_const.tile([1, 512], bf16, tag="onesrow")  # moving row for bias
    nc.vector.memset(onesrow[:], 1.0)
    ones512r = p_const.tile([1, 512], f32r, tag="ones512r")
    nc.vector.memset(stage_row[:], 1.0)
    nc.vector.tensor_copy(ones512r[:], stage_row[:])
    epscol2 = p_const.tile([1, 2], f32r, tag="epscol2")    # eps rides sq-stats MM
    nc.vector.memset(stage[:, 2:4], EPS)
    nc.vector.tensor_copy(epscol2[:], stage[0:1, 2:4])
    m01 = p_const.tile([128, 512], bf16, tag="m01")
    nc.sync.dma_start(out=m01, in_=m01_d)
    blm_t = p_const.tile([V, 1], f32, tag="blm")
    nc.sync.dma_start(out=blm_t, in_=blm_d.rearrange("(v o) -> v o", o=1))
    wlm_t = [p_const.tile([128, V], bf16, tag=f"wlm{kc}", name=f"wlm{kc}")
             for kc in range(KC)]
    for kc in range(KC):
        nc.sync.dma_start(out=wlm_t[kc], in_=wlm_d[kc * 128:(kc + 1) * 128, :])

    # ---- residual stream: one [128, 3*512] f32 tile per batch-pair ----
    x_t = [p_x.tile([128, KC * 512], f32r, tag=f"x{nt}", name=f"x{nt}")
           for nt in range(NBP)]
    for nt in range(NBP):
        for kc in range(KC):
            nc.sync.dma_start(out=x_t[nt][:, kc * 512:(kc + 1) * 512],
                              in_=x0T_d[kc * 128:(kc + 1) * 128,
                                        nt * 512:nt * 512 + 512])

    # ---- V_ext buffers: ones column written once, V slices per layer ----
    vext = [[p_v.tile([128, 2 * H * (HS + 1)], bf16, tag=f"vext{nt}_{bi}",
                      name=f"vext{nt}_{bi}") for bi in range(2)]
            for nt in range(NBP)]
    for nt in range(NBP):
        for bi in range(2):
            vxr = vext[nt][bi].rearrange("p (j h e) -> p j h e", j=2, h=H)
            nc.vector.memset(vxr[:, :, :, HS:HS + 1], 1.0)

    weights = {}

    def load_wqkv(l):
        w = weights.setdefault(l, {})
        w["wqkv"] = [p_w.tile([128, 3 * C], bf16, tag=f"wqkv{kc}",
                              name=f"wqkv{kc}", bufs=2) for kc in range(KC)]
        for kc in range(KC):
            nc.sync.dma_start(out=w["wqkv"][kc],
                              in_=wqkv_d[l, kc * 128:(kc + 1) * 128, :])
        w["bqkv"] = p_w.tile([128, 6], f32, tag="bqkv", name="bqkv", bufs=2)
        nc.sync.dma_start(out=w["bqkv"], in_=bqkv_d[l])

    def load_rest(l):
        w = weights.setdefault(l, {})
        w["wo"] = [p_w.tile([128, C], bf16, tag=f"wo{kc}", name=f"wo{kc}",
                            bufs=2) for kc in range(KC)]
        for kc in range(KC):
            nc.sync.dma_start(out=w["wo"][kc],
                              in_=wo_d[l, kc * 128:(kc + 1) * 128, :])
        w["w1"] = [p_w.tile([128, DFF], bf16, tag=f"w1{kc}", name=f"w1{kc}",
                            bufs=2) for kc in range(KC)]
        for kc in range(KC):
            nc.sync.dma_start(out=w["w1"][kc],
                              in_=w1_d[l, kc * 128:(kc + 1) * 128, :])
        w["b1"] = p_w.tile([128, K12], f32, tag="b1", name="b1", bufs=2)
        nc.sync.dma_start(out=w["b1"], in_=b1_d[l])
        w["w2"] = [p_w.tile([128, C], bf16, tag=f"w2_{kc}", name=f"w2_{kc}",
                            bufs=2) for kc in range(K12)]
        for kc in range(K12):
            nc.sync.dma_start(out=w["w2"][kc],
                              in_=w2_d[l, kc * 128:(kc + 1) * 128, :])
        w["brows"] = p_w.tile([1, 2 * C], bf16, tag="brows", name="brows",
                              bufs=2)
        nc.sync.dma_start(out=w["brows"], in_=brows_d[l])

    def stage_LN(nt, tag):
        """Standardize x_t[nt] -> new [128, 1536] bf16 tile.

        Stats via ones-matmuls; rsqrt as exp(-0.5*ln(var+eps)) so the whole
        kernel stays inside one ACT table set; per-token broadcasts via
        ones-column matmuls on the PE.
        """
        x = x_t[nt]
        xsq = p_big.tile([128, KC * 512], f32r, tag="xbig", name="xsq", bufs=2)
        nc.gpsimd.tensor_mul(xsq[:], x[:], x[:])
        mu_ps = ps_aux.tile([2, 512], f32, tag="aux", name="mu_ps")
        for kc in range(KC):
            nc.tensor.matmul(mu_ps[:], onesC[:],
                             x[:, kc * 512:(kc + 1) * 512],
                             start=(kc == 0), stop=(kc == KC - 1))
        sq_ps = ps_aux.tile([2, 512], f32, tag="aux", name="sq_ps")
        for kc in range(KC):
            nc.tensor.matmul(sq_ps[:], onesC[:],
                             xsq[:, kc * 512:(kc + 1) * 512],
                             start=(kc == 0), stop=False)
        nc.tensor.matmul(sq_ps[:], epscol2[:], ones512r[:], start=False,
                         stop=True)
        musq = p_rows.tile([1, 512], f32r, tag="musq", name="musq", bufs=1)
        nc.scalar.activation(musq[:], mu_ps[0:1, :], AF.Square, bias=0.0,
                             scale=1.0)
        # ve = E[x^2] + eps - mu^2
        ve = p_rows.tile([1, 512], f32, tag="ve", name="ve", bufs=2)
        nc.vector.tensor_tensor(out=ve[:], in0=sq_ps[0:1, :], in1=musq[:],
                                op=ALU.subtract)
        mu_sb = p_rows.tile([1, 512], f32r, tag="mu_sb", name="mu_sb", bufs=1)
        nc.scalar.copy(mu_sb[:], mu_ps[0:1, :])
        # rs = rsqrt(ve): quake seed + 2 Newton iterations, all on DVE so the
        # ACT engine never leaves the exp table set.
        ish = p_rows.tile([1, 512], i32, tag="ish", name="ish", bufs=1)
        nc.vector.tensor_scalar(out=ish[:], in0=ve[:].bitcast(i32), scalar1=1,
                                scalar2=None, op0=ALU.logical_shift_right)
        sdi = p_rows.tile([1, 512], i32, tag="sdi", name="sdi", bufs=1)
        nc.vector.tensor_scalar(out=sdi[:], in0=ish[:],
                                scalar1=float(0x5F3759DF), scalar2=-1.0,
                                op0=ALU.subtract, op1=ALU.mult)
        y0 = sdi[:].bitcast(f32)
        y1 = None
        for it_n in range(2):
            yv = y0 if it_n == 0 else y1[:]
            nt_a = p_rows.tile([1, 512], f32, tag="nt", name="nt", bufs=2)
            nc.vector.tensor_mul(nt_a[:], ve[:], yv)
            nt_b = p_rows.tile([1, 512], f32, tag="nt", name="nt", bufs=2)
            nc.vector.tensor_mul(nt_b[:], nt_a[:], yv)
            nt_c = p_rows.tile([1, 512], f32, tag="nc", name="nc", bufs=1)
            nc.vector.tensor_scalar(out=nt_c[:], in0=nt_b[:], scalar1=-0.5,
                                    scalar2=1.5, op0=ALU.mult, op1=ALU.add)
            dt_o = f32 if it_n == 0 else f32r
            tg = "y1" if it_n == 0 else "rs"
            y_n = p_rows.tile([1, 512], dt_o, tag=tg, name=tg, bufs=1)
            nc.vector.tensor_mul(y_n[:], nt_c[:], yv)
            y1 = y_n
        rs = y1
        mu_b = ps_aux.tile([128, 512], f32, tag="aux", name="mu_b")
        nc.tensor.matmul(mu_b[:], onescol[:], mu_sb[:], start=True, stop=True)
        rs_b = ps_aux.tile([128, 512], f32, tag="aux", name="rs_b")
        nc.tensor.matmul(rs_b[:], onescol[:], rs[:], start=True, stop=True)
        xn = p_xn.tile([128, KC * 512], bf16, tag=f"xn{nt}", name=f"xn{nt}",
                       bufs=1)
        for kc in range(KC):
            sl = slice(kc * 512, (kc + 1) * 512)
            xc = p_big.tile([128, 512], f32r, tag="xc", name="xc", bufs=2)
            nc.vector.tensor_tensor(out=xc[:], in0=x[:, sl], in1=mu_b[:],
                                    op=ALU.subtract)
            nc.vector.tensor_mul(xn[:, sl], xc[:], rs_b[:])
        return xn

    state = {}

    def stage_A(l, nt):
        state[nt] = {"xn": stage_LN(nt, "xn")}

    def stage_B(l, nt):
        w = weights[l]
        xn = state[nt]["xn"]
        qk = []
        for oc in range(6):
            qp = ps_big.tile([128, 512], f32, tag="big", name="qp")
            for kc in range(KC):
                nc.tensor.matmul(qp[:], w["wqkv"][kc][:, oc * 128:oc * 128 + 128],
                                 xn[:, kc * 512:(kc + 1) * 512],
                                 start=(kc == 0), stop=(kc == KC - 1))
            qt = p_qk.tile([128, 512], bf16, tag=f"qk{oc}", name=f"qk{oc}",
                           bufs=2)
            nc.scalar.activation(qt[:], qp[:], AF.Identity,
                                 bias=w["bqkv"][:, oc:oc + 1], scale=1.0)
            qk.append(qt)
        for bi in range(2):
            vxr = vext[nt][bi].rearrange("p (j h e) -> p j h e", j=2, h=H)
            for j in range(2):
                vp = ps_big.tile([128, C], f32, tag="big", name="vp")
                tc0 = bi * 256 + j * 128
                for kc in range(KC):
                    nc.tensor.matmul(vp[:], xn[:, kc * 512 + tc0:kc * 512 + tc0 + 128],
                                     w["wqkv"][kc][:, 2 * C:3 * C],
                                     start=(kc == 0), stop=(kc == KC - 1))
                nc.scalar.copy(vxr[:, j, :, 0:HS],
                               vp[:].rearrange("p (h d) -> p h d", h=H))
        state[nt]["qk"] = qk

    def stage_CD(l, nt):
        """Wave-pipelined scores -> exp -> mask -> attV -> normalize."""
        st = state[nt]
        qk = st["qk"]
        attc = [p_attc.tile([128, 512], bf16, tag=f"attc{kc}",
                            name=f"attc{kc}", bufs=3) for kc in range(KC)]
        ap_t = {}
        e_ms = {}
        LAG = 2
        for u in range(12 + LAG):
            if u < 12:
                h, bi = divmod(u, 2)
                qch, kch = h // 2, 3 + h // 2
                qrow = (h % 2) * 64
                q0 = bi * 256
                sp = ps_big.tile([128, 512], f32, tag="big", name="sp")
                qs = qk[qch][qrow:qrow + 64, q0:q0 + 256]
                nc.tensor.matmul(sp[:, 0:256],
                                 qk[kch][qrow:qrow + 64, q0:q0 + 128],
                                 qs, start=True, stop=True)
                nc.tensor.matmul(sp[:, 256:512],
                                 qk[kch][qrow:qrow + 64, q0 + 128:q0 + 256],
                                 qs, start=True, stop=True)
                e_t = p_e.tile([128, 512], bf16, tag="e_t", name="e_t", bufs=3)
                nc.scalar.activation(e_t[:], sp[:], AF.Exp, bias=0.0,
                                     scale=SCALE)
                e_m = p_e.tile([128, 512], bf16, tag="e_m", name="e_m", bufs=4)
                eng = nc.gpsimd if (u % 2 == 0) else nc.vector
                eng.tensor_mul(e_m[:], e_t[:], m01[:])
                e_ms[u] = e_m
            if u >= LAG:
                v = u - LAG
                h, bi = divmod(v, 2)
                qch = h // 2
                qrow = (h % 2) * 64
                q0 = bi * 256
                if bi == 0:
                    ap_t[h] = ps_ap.tile([HS + 1, 512], f32, tag="ap",
                                         name="ap_")
                ap_ = ap_t[h]
                e_m = e_ms.pop(v)
                vxr = vext[nt][bi].rearrange("p (j h e) -> p j h e", j=2, h=H)
                nc.tensor.matmul(ap_[:, q0:q0 + 256], vxr[:, 0, h, :],
                                 e_m[:, 0:256], start=True, stop=False)
                nc.tensor.matmul(ap_[:, q0:q0 + 256], vxr[:, 1, h, :],
                                 e_m[:, 256:512], start=False, stop=True)
                if bi == 1:
                    # 1/sum via DVE reciprocal, broadcast over 64 partitions
                    # via a K=1 ones-matmul, then one DVE multiply per head.
                    srow = p_rows.tile([1, 512], f32, tag="srow", name="srow",
                                       bufs=1)
                    nc.scalar.copy(srow[:], ap_[HS:HS + 1, :])
                    rec = p_rows.tile([1, 512], f32, tag="rec", name="rec",
                                      bufs=1)
                    nc.vector.reciprocal_approx_fast(out=rec[:], in_=srow[:])
                    rec_r = p_rows.tile([1, 512], f32r, tag="rec_r",
                                        name="rec_r", bufs=1)
                    nc.scalar.copy(rec_r[:], rec[:])
                    rec_b = ps_aux.tile([64, 512], f32, tag="aux",
                                        name="rec_b")
                    nc.tensor.matmul(rec_b[:], onescol[:, 0:64], rec_r[:],
                                     start=True, stop=True)
                    rb_sb = p_rows.tile([64, 512], bf16, tag="rb_sb",
                                        name="rb_sb", bufs=2)
                    nc.scalar.copy(rb_sb[:], rec_b[:])
                    qr2 = (h % 2) * 64
                    nc.vector.tensor_mul(attc[qch][qr2:qr2 + 64, :],
                                         ap_[0:HS, :], rb_sb[:])
        state[nt]["attc"] = attc
        del state[nt]["qk"], state[nt]["xn"]

    def stage_E(l, nt):
        w = weights[l]
        attc = state[nt]["attc"]
        for oc in range(KC):
            wp = ps_big.tile([128, 512], f32, tag="big", name="wp")
            nc.tensor.matmul(wp[:], w["brows"][0:1, oc * 128:oc * 128 + 128],
                             onesrow[:], start=True, stop=False)
            for kc in range(KC):
                nc.tensor.matmul(wp[:], w["wo"][kc][:, oc * 128:oc * 128 + 128],
                                 attc[kc][:], start=False, stop=(kc == KC - 1))
            sl = slice(oc * 512, (oc + 1) * 512)
            nc.vector.tensor_tensor(out=x_t[nt][:, sl], in0=wp[:],
                                    in1=x_t[nt][:, sl], op=ALU.add)
        del state[nt]["attc"]

    def stage_F(l, nt):
        state[nt]["h2n"] = stage_LN(nt, "h2n")

    def stage_G(l, nt):
        w = weights[l]
        h2n = state[nt]["h2n"]
        a_t = []
        for kc12 in range(K12):
            fp1 = ps_big.tile([128, 512], f32, tag="big", name="fp1")
            for kc in range(KC):
                nc.tensor.matmul(fp1[:],
                                 w["w1"][kc][:, kc12 * 128:kc12 * 128 + 128],
                                 h2n[:, kc * 512:(kc + 1) * 512],
                                 start=(kc == 0), stop=(kc == KC - 1))
            a = p_a.tile([128, 512], bf16, tag=f"a{kc12}", name=f"a{kc12}",
                         bufs=1)
            nc.scalar.activation(a[:], fp1[:], AF.Relu,
                                 bias=w["b1"][:, kc12:kc12 + 1], scale=1.0)
            a_t.append(a)
        state[nt]["a"] = a_t
        del state[nt]["h2n"]

    def stage_H(l, nt):
        w = weights[l]
        a_t = state[nt]["a"]
        for oc in range(KC):
            fp2 = ps_big.tile([128, 512], f32, tag="big", name="fp2")
            nc.tensor.matmul(fp2[:],
                             w["brows"][0:1, C + oc * 128:C + oc * 128 + 128],
                             onesrow[:], start=True, stop=False)
            for kc12 in range(K12):
                nc.tensor.matmul(fp2[:],
                                 w["w2"][kc12][:, oc * 128:oc * 128 + 128],
                                 a_t[kc12][:], start=False,
                                 stop=(kc12 == K12 - 1))
            sl = slice(oc * 512, (oc + 1) * 512)
            nc.vector.tensor_tensor(out=x_t[nt][:, sl], in0=fp2[:],
                                    in1=x_t[nt][:, sl], op=ALU.add)
        del state[nt]

    def stage_HEAD(nt):
        xf = stage_LN(nt, "xf")
        lp = ps_big.tile([V, 512], f32, tag="big", name="lp")
        for kc in range(KC):
            nc.tensor.matmul(lp[:], wlm_t[kc][:],
                             xf[:, kc * 512:(kc + 1) * 512],
                             start=(kc == 0), stop=(kc == KC - 1))
        osb = p_out.tile([V, 512], f32, tag="osb", name="osb")
        nc.scalar.activation(osb[:], lp[:], AF.Identity, bias=blm_t[:],
                             scale=1.0)
        nc.sync.dma_start(out=outT_d[:, nt * 512:nt * 512 + 512], in_=osb[:])

    # ---- stage-major emission: 4 independent batch-pair streams per stage ----
    load_wqkv(0)
    load_rest(0)
    for nt in range(NBP):
        stage_A(0, nt)
    for l in range(N_LAYERS):
        for nt in range(NBP):
            stage_B(l, nt)
            stage_CD(l, nt)
        if l + 1 < N_LAYERS:
            load_wqkv(l + 1)
        for nt in range(NBP):
            stage_E(l, nt)
            stage_F(l, nt)
        if l + 1 < N_LAYERS:
            load_rest(l + 1)
        for nt in range(NBP):
            stage_G(l, nt)
            stage_H(l, nt)
            if l + 1 < N_LAYERS:
                stage_A(l + 1, nt)
            else:
                stage_HEAD(nt)

    ctx.close()


def _host_prep(inputs):
    """Fold LN affine params into weights; build per-core input maps."""
    f = lambda k: np.asarray(inputs[k], dtype=np.float32)
    tobf = lambda a: np.ascontiguousarray(a.astype(ml_dtypes.bfloat16))
    idx = np.asarray(inputs["idx"]).astype(np.int64)
    tok_emb, pos_emb = f("tok_emb"), f("pos_emb")
    Wq, Wk, Wv, Wo = f("Wq"), f("Wk"), f("Wv"), f("Wo")
    bo, W1, b1, W2, b2 = f("bo"), f("W1"), f("b1"), f("W2"), f("b2")
    ln1_g, ln1_b = f("ln1_g"), f("ln1_b")
    ln2_g, ln2_b = f("ln2_g"), f("ln2_b")
    lnf_g, lnf_b = f("lnf_g"), f("lnf_b")
    Wlm, blm = f("Wlm"), f("blm")

    # [L,H,C,HS] -> [L,C,H*HS]
    Wq_all = np.transpose(Wq, (0, 2, 1, 3)).reshape(L, C, C)
    Wk_all = np.transpose(Wk, (0, 2, 1, 3)).reshape(L, C, C)
    Wv_all = np.transpose(Wv, (0, 2, 1, 3)).reshape(L, C, C)

    g1 = ln1_g[:, :, None]
    wqkv = np.concatenate([g1 * Wq_all, g1 * Wk_all, g1 * Wv_all], axis=2)
    bq = np.einsum("lc,lcd->ld", ln1_b, Wq_all)
    bk = np.einsum("lc,lcd->ld", ln1_b, Wk_all)
    bv = np.einsum("lc,lcd->ld", ln1_b, Wv_all)
    bo2 = bo + np.einsum("ld,ldc->lc", bv, Wo)       # v-bias folds through Wo
    w1f = ln2_g[:, :, None] * W1
    b1f = b1 + np.einsum("lc,lcd->ld", ln2_b, W1)
    wlmf = lnf_g[:, None] * Wlm
    blmf = blm + lnf_b @ Wlm

    bqkv = np.concatenate([bq, bk], axis=1)          # [L, 768]
    bqkv_cols = np.ascontiguousarray(
        bqkv.reshape(L, 6, 128).transpose(0, 2, 1)).astype(np.float32)
    b1_cols = np.ascontiguousarray(
        b1f.reshape(L, K12, 128).transpose(0, 2, 1)).astype(np.float32)
    brows = tobf(np.concatenate([bo2, b2], axis=1)[:, None, :])  # [L,1,2C]

    # multiplicative causal mask, key-major: cols = (key_block, q)
    p = np.arange(128)[:, None]
    q = np.arange(256)[None, :]
    m0 = (p <= q).astype(np.float32)          # keys 0..127
    m1 = (p + 128 <= q).astype(np.float32)    # keys 128..255
    m01 = tobf(np.concatenate([m0, m1], axis=1))    # [128, 512]

    x0 = tok_emb[idx] + pos_emb[None]                # [B,T,C] f32
    in_maps = []
    for c in range(N_CORES):
        x0c = x0[c * BPC:(c + 1) * BPC].reshape(NTOK, C)
        in_maps.append({
            "x0T": np.ascontiguousarray(x0c.T),
            "wqkv": tobf(wqkv),
            "bqkv": bqkv_cols,
            "wo": tobf(Wo),
            "w1": tobf(w1f),
            "b1": b1_cols,
            "w2": tobf(W2),
            "brows": brows,
            "wlm": tobf(wlmf),
            "blm": np.ascontiguousarray(blmf),
            "m01": m01,
        })
    return in_maps


def _run(inputs, trace=False):
    if "nc" not in _cache:
        _cache["nc"] = _build_nc()
    nc = _cache["nc"]
    in_maps = _host_prep(inputs)
    res = run_bass_kernel_spmd(nc, in_maps, core_ids=list(range(N_CORES)),
                               trace=trace)
    outs = []
    for c in range(N_CORES):
        outT = res.results[c]["outT"]                 # [V, NTOK]
        outs.append(outT.T.reshape(BPC, T, V))
    logits = np.concatenate(outs, axis=0).astype(np.float32)
    return logits, res


def kernel(**inputs) -> np.ndarray:
    logits, _ = _run(inputs, trace=False)
    return logits


# revision 23
# speedup vs baseline: 1.1612x; 1.1612x over previous
"""Bass/Trainium2 kernel for nn_CharLevelLanguageModel (6-layer char transformer).

v2 strategy: data-parallel over batch (64 -> 8 cores x 8). Per core, each layer
is emitted stage-major across the 4 batch-pair (512-token) tiles so that every
serial chain (LN row ops, softmax normalize) is covered by independent matmul
work from the other batch-pairs.

Key choices vs v1:
- bf16 matmul operands (weights + activations); residual stream and stats stay
  fp32 (f32r) in SBUF; PSUM accumulates fp32.
- Zero GpSimd instructions. Partition broadcasts are ones-column matmuls on the
  PE; elementwise work is split between DVE and ACT.
- Single ACT table set ("natural_log_exp_and_others"): rsqrt(v)=exp(-0.5*ln v),
  1/s = exp(-ln s). No table reloads after startup.
- LN affine params folded into adjacent weights on host; biases ride K=1
  bf16 matmul rows (brows x onesrow) or ACT bias columns.
- Causal mask is one multiplicative bf16 DVE op per (head, batch) unit.
"""

import os
import numpy as np
import ml_dtypes

import concourse.bass as bass
import concourse.mybir as mybir
import concourse.tile as tile
from concourse import bacc
from concourse.bass_utils import run_bass_kernel_spmd

B, T, C, H, L, V = 64, 256, 384, 6, 6, 65
HS = C // H          # 64
DFF = 4 * C          # 1536
N_CORES = 8
BPC = B // N_CORES   # 8 batches per core
NTOK = BPC * T       # 2048 tokens per core
NBP = 4              # batch-pair (512-token) tiles per core
KC = C // 128        # 3 feature chunks
K12 = DFF // 128     # 12 dff chunks
EPS = 1e-5
SCALE = HS ** -0.5

f32 = mybir.dt.float32
f32r = mybir.dt.float32r
bf16 = mybir.dt.bfloat16
i32 = mybir.dt.int32
AF = mybir.ActivationFunctionType
ALU = mybir.AluOpType

N_LAYERS = int(os.environ.get("KERNEL_LAYERS", str(L)))

_cache = {}


def _build_nc():
    nc = bacc.Bacc("TRN2", target_bir_lowering=False, debug=False,
                   num_devices=N_CORES)

    x0T_d = nc.dram_tensor("x0T", [C, NTOK], f32r, kind="ExternalInput").ap()
    wqkv_d = nc.dram_tensor("wqkv", [L, C, 3 * C], bf16, kind="ExternalInput").ap()
    bqkv_d = nc.dram_tensor("bqkv", [L, 128, 6], f32, kind="ExternalInput").ap()
    wo_d = nc.dram_tensor("wo", [L, C, C], bf16, kind="ExternalInput").ap()
    w1_d = nc.dram_tensor("w1", [L, C, DFF], bf16, kind="ExternalInput").ap()
    b1_d = nc.dram_tensor("b1", [L, 128, K12], f32, kind="ExternalInput").ap()
    w2_d = nc.dram_tensor("w2", [L, DFF, C], bf16, kind="ExternalInput").ap()
    brows_d = nc.dram_tensor("brows", [L, 1, 2 * C], bf16, kind="ExternalInput").ap()
    wlm_d = nc.dram_tensor("wlm", [C, V], bf16, kind="ExternalInput").ap()
    blm_d = nc.dram_tensor("blm", [V], f32, kind="ExternalInput").ap()
    m01_d = nc.dram_tensor("m01", [128, 512], bf16, kind="ExternalInput").ap()
    outT_d = nc.dram_tensor("outT", [V, NTOK], f32, kind="ExternalOutput").ap()

    with tile.TileContext(nc) as tc:
        _build_body(nc, tc, x0T_d, wqkv_d, bqkv_d, wo_d, w1_d, b1_d, w2_d,
                    brows_d, wlm_d, blm_d, m01_d, outT_d)
    nc.compile()
    return nc


def _build_body(nc, tc, x0T_d, wqkv_d, bqkv_d, wo_d, w1_d, b1_d, w2_d,
                brows_d, wlm_d, blm_d, m01_d, outT_d):
    import contextlib
    ctx = contextlib.ExitStack()
    p_const = ctx.enter_context(tc.tile_pool(name="consts", bufs=1))
    p_x = ctx.enter_context(tc.tile_pool(name="x", bufs=1))
    p_w = ctx.enter_context(tc.tile_pool(name="w", bufs=1))
    p_big = ctx.enter_context(tc.tile_pool(name="bigsb", bufs=1))   # xsq/xc
    p_xn = ctx.enter_context(tc.tile_pool(name="xn", bufs=1))
    p_rows = ctx.enter_context(tc.tile_pool(name="rows", bufs=1))
    p_qk = ctx.enter_context(tc.tile_pool(name="qk", bufs=1))
    p_v = ctx.enter_context(tc.tile_pool(name="v", bufs=1))
    p_e = ctx.enter_context(tc.tile_pool(name="e", bufs=1))
    p_attc = ctx.enter_context(tc.tile_pool(name="attc", bufs=1))
    p_a = ctx.enter_context(tc.tile_pool(name="a", bufs=1))
    p_out = ctx.enter_context(tc.tile_pool(name="out", bufs=2))
    ps_aux = ctx.enter_context(tc.tile_pool(name="ps_aux", bufs=2, space="PSUM"))
    ps_ap = ctx.enter_context(tc.tile_pool(name="ps_ap", bufs=2, space="PSUM"))
    ps_big = ctx.enter_context(tc.tile_pool(name="ps_big", bufs=4, space="PSUM"))

    # ---- constants ----
    stage = p_const.tile([128, 8], f32, tag="stage")
    onesC = p_const.tile([128, 2], f32r, tag="onesC")      # 1/C for mean matmuls
    nc.vector.memset(stage[:, 0:2], 1.0 / C)
    nc.vector.tensor_copy(onesC[:], stage[:, 0:2])
    stage_row = p_const.tile([1, 512], f32, tag="stage_row")
    nc.vector.memset(stage_row[:, 0:128], 1.0)
    onescol = p_const.tile([1, 128], f32r, tag="onescol")  # bcast lhsT
    nc.vector.tensor_copy(onescol[:], stage_row[:, 0:128])
    onescol34 = p_const.tile([34, 128], f32r, tag="onescol34")
    nc.vector.tensor_copy(onescol34[0:1, :], stage_row[:, 0:128])
    nc.vector.tensor_copy(onescol34[32:33, :], stage_row[:, 0:128])
    onesrow = p_const.tile([1, 512], bf16, tag="onesrow")  # moving row for bias
    nc.vector.memset(onesrow[:], 1.0)
    ones512r = p_const.tile([1, 512], f32r, tag="ones512r")
    nc.vector.memset(stage_row[:], 1.0)
    nc.vector.tensor_copy(ones512r[:], stage_row[:])
    epscol2 = p_const.tile([1, 2], f32r, tag="epscol2")    # eps rides sq-stats MM
    nc.vector.memset(stage[:, 2:4], EPS)
    nc.vector.tensor_copy(epscol2[:], stage[0:1, 2:4])
    m01 = p_const.tile([128, 512], bf16, tag="m01")
    nc.sync.dma_start(out=m01, in_=m01_d)
    blm_t = p_const.tile([V, 1], f32, tag="blm")
    nc.sync.dma_start(out=blm_t, in_=blm_d.rearrange("(v o) -> v o", o=1))
    wlm_t = [p_const.tile([128, V], bf16, tag=f"wlm{kc}", name=f"wlm{kc}")
             for kc in range(KC)]
    for kc in range(KC):
        nc.sync.dma_start(out=wlm_t[kc], in_=wlm_d[kc * 128:(kc + 1) * 128, :])

    # ---- residual stream: one [128, 3*512] f32 tile per batch-pair ----
    x_t = [p_x.tile([128, KC * 512], f32r, tag=f"x{nt}", name=f"x{nt}")
           for nt in range(NBP)]
    for nt in range(NBP):
        for kc in range(KC):
            nc.sync.dma_start(out=x_t[nt][:, kc * 512:(kc + 1) * 512],
                              in_=x0T_d[kc * 128:(kc + 1) * 128,
                                        nt * 512:nt * 512 + 512])

    # ---- V_ext buffers: ones column written once, V slices per layer ----
    vext = [[p_v.tile([128, 2 * H * (HS + 1)], bf16, tag=f"vext{nt}_{bi}",
                      name=f"vext{nt}_{bi}") for bi in range(2)]
            for nt in range(NBP)]
    for nt in range(NBP):
        for bi in range(2):
            vxr = vext[nt][bi].rearrange("p (j h e) -> p j h e", j=2, h=H)
            nc.vector.memset(vxr[:, :, :, HS:HS + 1], 1.0)

    weights = {}

    def load_wqkv(l):
        w = weights.setdefault(l, {})
        w["wqkv"] = [p_w.tile([128, 3 * C], bf16, tag=f"wqkv{kc}",
                              name=f"wqkv{kc}", bufs=2) for kc in range(KC)]
        for kc in range(KC):
            nc.sync.dma_start(out=w["wqkv"][kc],
                              in_=wqkv_d[l, kc * 128:(kc + 1) * 128, :])
        w["bqkv"] = p_w.tile([128, 6], f32, tag="bqkv", name="bqkv", bufs=2)
        nc.sync.dma_start(out=w["bqkv"], in_=bqkv_d[l])

    def load_rest(l):
        w = weights.setdefault(l, {})
        w["wo"] = [p_w.tile([128, C], bf16, tag=f"wo{kc}", name=f"wo{kc}",
                            bufs=2) for kc in range(KC)]
        for kc in range(KC):
            nc.sync.dma_start(out=w["wo"][kc],
                              in_=wo_d[l, kc * 128:(kc + 1) * 128, :])
        w["w1"] = [p_w.tile([128, DFF], bf16, tag=f"w1{kc}", name=f"w1{kc}",
                            bufs=2) for kc in range(KC)]
        for kc in range(KC):
            nc.sync.dma_start(out=w["w1"][kc],
                              in_=w1_d[l, kc * 128:(kc + 1) * 128, :])
        w["b1"] = p_w.tile([128, K12], f32, tag="b1", name="b1", bufs=2)
        nc.sync.dma_start(out=w["b1"], in_=b1_d[l])
        w["w2"] = [p_w.tile([128, C], bf16, tag=f"w2_{kc}", name=f"w2_{kc}",
                            bufs=2) for kc in range(K12)]
        for kc in range(K12):
            nc.sync.dma_start(out=w["w2"][kc],
                              in_=w2_d[l, kc * 128:(kc + 1) * 128, :])
        w["brows"] = p_w.tile([1, 2 * C], bf16, tag="brows", name="brows",
                              bufs=2)
        nc.sync.dma_start(out=w["brows"], in_=brows_d[l])

    def stage_LN(nts):
        """Standardize x_t[nt] for each nt in `nts` (a pair of batch-pairs).

        Stats via ones-matmuls; the row chain (quake rsqrt seed + 2 Newton
        steps, all on DVE) is batched across the pair on a [34,512] tile
        (rows 0 and 32), since DVE small-op cost is per-instruction, not
        per-row. Per-token broadcasts are ones-column matmuls on the PE.
        Returns {nt: xn}.
        """
        P = 32 * (len(nts) - 1) + 2
        statps = {}
        for nt in nts:
            x = x_t[nt]
            xsq = p_big.tile([128, KC * 512], f32r, tag="xbig", name="xsq",
                             bufs=2)
            nc.vector.tensor_mul(xsq[:], x[:], x[:])
            mu_ps = ps_big.tile([2, 512], f32, tag="big", name="mu_ps")
            for kc in range(KC):
                nc.tensor.matmul(mu_ps[:], onesC[:],
                                 x[:, kc * 512:(kc + 1) * 512],
                                 start=(kc == 0), stop=(kc == KC - 1))
            sq_ps = ps_big.tile([2, 512], f32, tag="big", name="sq_ps")
            for kc in range(KC):
                nc.tensor.matmul(sq_ps[:], onesC[:],
                                 xsq[:, kc * 512:(kc + 1) * 512],
                                 start=(kc == 0), stop=False)
            nc.tensor.matmul(sq_ps[:], epscol2[:], ones512r[:], start=False,
                             stop=True)
            statps[nt] = (mu_ps, sq_ps)
        mu_sb = p_rows.tile([P, 512], f32r, tag="mu_sb", name="mu_sb", bufs=1)
        for i, nt in enumerate(nts):
            nc.scalar.copy(mu_sb[32 * i:32 * i + 1, :], statps[nt][0][0:1, :])
        musq = p_rows.tile([P, 512], f32r, tag="musq", name="musq", bufs=1)
        nc.scalar.activation(musq[:], mu_sb[:], AF.Square, bias=0.0, scale=1.0)
        # ve = E[x^2] + eps - mu^2   (junk in rows 2..31 is never read)
        ve = p_rows.tile([P, 512], f32, tag="ve", name="ve", bufs=2)
        for i, nt in enumerate(nts):
            nc.vector.tensor_tensor(out=ve[32 * i:32 * i + 1, :],
                                    in0=statps[nt][1][0:1, :],
                                    in1=musq[32 * i:32 * i + 1, :],
                                    op=ALU.subtract)
        ish = p_rows.tile([P, 512], i32, tag="ish", name="ish", bufs=1)
        nc.vector.tensor_scalar(out=ish[:], in0=ve[:].bitcast(i32), scalar1=1,
                                scalar2=None, op0=ALU.logical_shift_right)
        sdi = p_rows.tile([P, 512], i32, tag="sdi", name="sdi", bufs=1)
        nc.vector.tensor_scalar(out=sdi[:], in0=ish[:],
                                scalar1=float(0x5F3759DF), scalar2=-1.0,
                                op0=ALU.subtract, op1=ALU.mult)
        y0 = sdi[:].bitcast(f32)
        y1 = None
        for it_n in range(2):
            yv = y0 if it_n == 0 else y1[:]
            nt_a = p_rows.tile([P, 512], f32, tag="nt", name="nt", bufs=2)
            nc.vector.tensor_mul(nt_a[:], ve[:], yv)
            nt_b = p_rows.tile([P, 512], f32, tag="nt", name="nt", bufs=2)
            nc.vector.tensor_mul(nt_b[:], nt_a[:], yv)
            nt_c = p_rows.tile([P, 512], f32, tag="nc", name="nc", bufs=1)
            nc.vector.tensor_scalar(out=nt_c[:], in0=nt_b[:], scalar1=-0.5,
                                    scalar2=1.5, op0=ALU.mult, op1=ALU.add)
            dt_o = f32 if it_n == 0 else f32r
            tg = "y1" if it_n == 0 else "rs"
            y_n = p_rows.tile([P, 512], dt_o, tag=tg, name=tg, bufs=1)
            nc.vector.tensor_mul(y_n[:], nt_c[:], yv)
            y1 = y_n
        rs = y1
        xns = {}
        for i, nt in enumerate(nts):
            oc_i = onescol34[32 * i:32 * i + 1, :]
            mu_b = ps_aux.tile([128, 512], f32, tag="aux", name="mu_b")
            nc.tensor.matmul(mu_b[:], oc_i, mu_sb[32 * i:32 * i + 1, :],
                             start=True, stop=True)
            rs_b = ps_aux.tile([128, 512], f32, tag="aux", name="rs_b")
            nc.tensor.matmul(rs_b[:], oc_i, rs[32 * i:32 * i + 1, :],
                             start=True, stop=True)
            xn = p_xn.tile([128, KC * 512], bf16, tag=f"xn{nt}",
                           name=f"xn{nt}", bufs=1)
            for kc in range(KC):
                sl = slice(kc * 512, (kc + 1) * 512)
                xc = p_big.tile([128, 512], f32r, tag="xc", name="xc", bufs=2)
                nc.vector.tensor_tensor(out=xc[:], in0=x_t[nt][:, sl],
                                        in1=mu_b[:], op=ALU.subtract)
                nc.vector.tensor_mul(xn[:, sl], xc[:], rs_b[:])
            xns[nt] = xn
        return xns

    state = {}

    def stage_A(l, nts):
        xns = stage_LN(nts)
        for nt in nts:
            state[nt] = {"xn": xns[nt]}

    def stage_B(l, nt):
        w = weights[l]
        xn = state[nt]["xn"]
        qk = []
        for oc in range(6):
            qp = ps_big.tile([128, 512], f32, tag="big", name="qp")
            for kc in range(KC):
                nc.tensor.matmul(qp[:], w["wqkv"][kc][:, oc * 128:oc * 128 + 128],
                                 xn[:, kc * 512:(kc + 1) * 512],
                                 start=(kc == 0), stop=(kc == KC - 1))
            qt = p_qk.tile([128, 512], bf16, tag=f"qk{oc}", name=f"qk{oc}",
                           bufs=2)
            nc.scalar.activation(qt[:], qp[:], AF.Identity,
                                 bias=w["bqkv"][:, oc:oc + 1], scale=1.0)
            qk.append(qt)
        for bi in range(2):
            vxr = vext[nt][bi].rearrange("p (j h e) -> p j h e", j=2, h=H)
            for j in range(2):
                vp = ps_big.tile([128, C], f32, tag="big", name="vp")
                tc0 = bi * 256 + j * 128
                for kc in range(KC):
                    nc.tensor.matmul(vp[:], xn[:, kc * 512 + tc0:kc * 512 + tc0 + 128],
                                     w["wqkv"][kc][:, 2 * C:3 * C],
                                     start=(kc == 0), stop=(kc == KC - 1))
                nc.scalar.copy(vxr[:, j, :, 0:HS],
                               vp[:].rearrange("p (h d) -> p h d", h=H))
        state[nt]["qk"] = qk

    def stage_CD(l, nt):
        """Wave-pipelined scores -> exp -> mask -> attV -> normalize."""
        st = state[nt]
        qk = st["qk"]
        attc = [p_attc.tile([128, 512], bf16, tag=f"attc{kc}",
                            name=f"attc{kc}", bufs=3) for kc in range(KC)]
        ap_t = {}
        e_ms = {}
        LAG = 2
        for u in range(12 + LAG):
            if u < 12:
                h, bi = divmod(u, 2)
                qch, kch = h // 2, 3 + h // 2
                qrow = (h % 2) * 64
                q0 = bi * 256
                sp = ps_big.tile([128, 512], f32, tag="big", name="sp")
                qs = qk[qch][qrow:qrow + 64, q0:q0 + 256]
                nc.tensor.matmul(sp[:, 0:256],
                                 qk[kch][qrow:qrow + 64, q0:q0 + 128],
                                 qs, start=True, stop=True)
                nc.tensor.matmul(sp[:, 256:512],
                                 qk[kch][qrow:qrow + 64, q0 + 128:q0 + 256],
                                 qs, start=True, stop=True)
                e_t = p_e.tile([128, 512], bf16, tag="e_t", name="e_t", bufs=3)
                nc.scalar.activation(e_t[:], sp[:], AF.Exp, bias=0.0,
                                     scale=SCALE)
                e_m = p_e.tile([128, 512], bf16, tag="e_m", name="e_m", bufs=4)
                nc.vector.tensor_mul(e_m[:], e_t[:], m01[:])
                e_ms[u] = e_m
            if u >= LAG:
                v = u - LAG
                h, bi = divmod(v, 2)
                qch = h // 2
                qrow = (h % 2) * 64
                q0 = bi * 256
                if bi == 0:
                    ap_t[h] = ps_ap.tile([HS + 1, 512], f32, tag="ap",
                                         name="ap_")
                ap_ = ap_t[h]
                e_m = e_ms.pop(v)
                vxr = vext[nt][bi].rearrange("p (j h e) -> p j h e", j=2, h=H)
                nc.tensor.matmul(ap_[:, q0:q0 + 256], vxr[:, 0, h, :],
                                 e_m[:, 0:256], start=True, stop=False)
                nc.tensor.matmul(ap_[:, q0:q0 + 256], vxr[:, 1, h, :],
                                 e_m[:, 256:512], start=False, stop=True)
                if bi == 1:
                    # 1/sum batched per head pair: copy both sum rows into a
                    # [34,512] tile (rows 0/32), one DVE reciprocal + one f32r
                    # rounding copy, then a 64-row broadcast matmul and one
                    # DVE multiply per head.
                    if h % 2 == 0:
                        srow = p_rows.tile([34, 512], f32, tag="srow",
                                           name="srow", bufs=1)
                        state[nt]["srow"] = srow
                        nc.scalar.copy(srow[0:1, :], ap_[HS:HS + 1, :])
                    else:
                        srow = state[nt]["srow"]
                        nc.scalar.copy(srow[32:33, :], ap_[HS:HS + 1, :])
                        rec = p_rows.tile([34, 512], f32, tag="rec",
                                          name="rec", bufs=1)
                        nc.vector.reciprocal_approx_fast(out=rec[:],
                                                         in_=srow[:])
                        rec_r = p_rows.tile([34, 512], f32r, tag="rec_r",
                                            name="rec_r", bufs=1)
                        nc.scalar.copy(rec_r[:], rec[:])
                        for hh in (h - 1, h):
                            rec_b = ps_aux.tile([64, 512], f32, tag="aux",
                                                name="rec_b")
                            bb = 32 * (hh % 2)
                            nc.tensor.matmul(rec_b[:],
                                             onescol34[bb:bb + 1, 0:64],
                                             rec_r[bb:bb + 1, :],
                                             start=True, stop=True)
                            rb_sb = p_rows.tile([64, 512], bf16, tag="rb_sb",
                                                name="rb_sb", bufs=2)
                            nc.scalar.copy(rb_sb[:], rec_b[:])
                            nc.vector.tensor_mul(
                                attc[qch][(hh % 2) * 64:(hh % 2) * 64 + 64, :],
                                ap_t[hh][0:HS, :], rb_sb[:])
        state[nt]["attc"] = attc
        del state[nt]["qk"], state[nt]["xn"], state[nt]["srow"]

    def stage_E(l, nt):
        w = weights[l]
        attc = state[nt]["attc"]
        for oc in range(KC):
            wp = ps_big.tile([128, 512], f32, tag="big", name="wp")
            nc.tensor.matmul(wp[:], w["brows"][0:1, oc * 128:oc * 128 + 128],
                             onesrow[:], start=True, stop=False)
            for kc in range(KC):
                nc.tensor.matmul(wp[:], w["wo"][kc][:, oc * 128:oc * 128 + 128],
                                 attc[kc][:], start=False, stop=(kc == KC - 1))
            sl = slice(oc * 512, (oc + 1) * 512)
            nc.vector.tensor_tensor(out=x_t[nt][:, sl], in0=wp[:],
                                    in1=x_t[nt][:, sl], op=ALU.add)
        del state[nt]["attc"]

    def stage_F(l, nts):
        xns = stage_LN(nts)
        for nt in nts:
            state[nt]["h2n"] = xns[nt]

    def stage_G(l, nt):
        w = weights[l]
        h2n = state[nt]["h2n"]
        a_t = []
        for kc12 in range(K12):
            fp1 = ps_big.tile([128, 512], f32, tag="big", name="fp1")
            for kc in range(KC):
                nc.tensor.matmul(fp1[:],
                                 w["w1"][kc][:, kc12 * 128:kc12 * 128 + 128],
                                 h2n[:, kc * 512:(kc + 1) * 512],
                                 start=(kc == 0), stop=(kc == KC - 1))
            a = p_a.tile([128, 512], bf16, tag=f"a{kc12}", name=f"a{kc12}",
                         bufs=1)
            nc.scalar.activation(a[:], fp1[:], AF.Relu,
                                 bias=w["b1"][:, kc12:kc12 + 1], scale=1.0)
            a_t.append(a)
        state[nt]["a"] = a_t
        del state[nt]["h2n"]

    def stage_H(l, nt):
        w = weights[l]
        a_t = state[nt]["a"]
        for oc in range(KC):
            fp2 = ps_big.tile([128, 512], f32, tag="big", name="fp2")
            nc.tensor.matmul(fp2[:],
                             w["brows"][0:1, C + oc * 128:C + oc * 128 + 128],
                             onesrow[:], start=True, stop=False)
            for kc12 in range(K12):
                nc.tensor.matmul(fp2[:],
                                 w["w2"][kc12][:, oc * 128:oc * 128 + 128],
                                 a_t[kc12][:], start=False,
                                 stop=(kc12 == K12 - 1))
            sl = slice(oc * 512, (oc + 1) * 512)
            nc.vector.tensor_tensor(out=x_t[nt][:, sl], in0=fp2[:],
                                    in1=x_t[nt][:, sl], op=ALU.add)
        del state[nt]

    def stage_HEAD(nts):
        xfs = stage_LN(nts)
        for nt in nts:
            xf = xfs[nt]
            lp = ps_big.tile([V, 512], f32, tag="big", name="lp")
            for kc in range(KC):
                nc.tensor.matmul(lp[:], wlm_t[kc][:],
                                 xf[:, kc * 512:(kc + 1) * 512],
                                 start=(kc == 0), stop=(kc == KC - 1))
            osb = p_out.tile([V, 512], f32, tag="osb", name="osb")
            nc.scalar.activation(osb[:], lp[:], AF.Identity, bias=blm_t[:],
                                 scale=1.0)
            nc.sync.dma_start(out=outT_d[:, nt * 512:nt * 512 + 512],
                              in_=osb[:])

    # ---- stage-major emission: 4 independent batch-pair streams per stage ----
    load_wqkv(0)
    load_rest(0)
    stage_A(0, [0, 1])
    stage_A(0, [2, 3])
    for l in range(N_LAYERS):
        for nt in range(NBP):
            stage_B(l, nt)
            stage_CD(l, nt)
        if l + 1 < N_LAYERS:
            load_wqkv(l + 1)
        stage_E(l, 0)
        stage_E(l, 1)
        stage_F(l, [0, 1])
        stage_E(l, 2)
        stage_E(l, 3)
        stage_F(l, [2, 3])
        if l + 1 < N_LAYERS:
            load_rest(l + 1)
        for pair in ([0, 1], [2, 3]):
            for nt in pair:
                stage_G(l, nt)
                stage_H(l, nt)
            if l + 1 < N_LAYERS:
                stage_A(l + 1, pair)
            else:
                stage_HEAD(pair)

    ctx.close()


def _host_prep(inputs):
    """Fold LN affine params into weights; build per-core input maps."""
    f = lambda k: np.asarray(inputs[k], dtype=np.float32)
    tobf = lambda a: np.ascontiguousarray(a.astype(ml_dtypes.bfloat16))
    idx = np.asarray(inputs["idx"]).astype(np.int64)
    tok_emb, pos_emb = f("tok_emb"), f("pos_emb")
    Wq, Wk, Wv, Wo = f("Wq"), f("Wk"), f("Wv"), f("Wo")
    bo, W1, b1, W2, b2 = f("bo"), f("W1"), f("b1"), f("W2"), f("b2")
    ln1_g, ln1_b = f("ln1_g"), f("ln1_b")
    ln2_g, ln2_b = f("ln2_g"), f("ln2_b")
    lnf_g, lnf_b = f("lnf_g"), f("lnf_b")
    Wlm, blm = f("Wlm"), f("blm")

    # [L,H,C,HS] -> [L,C,H*HS]
    Wq_all = np.transpose(Wq, (0, 2, 1, 3)).reshape(L, C, C)
    Wk_all = np.transpose(Wk, (0, 2, 1, 3)).reshape(L, C, C)
    Wv_all = np.transpose(Wv, (0, 2, 1, 3)).reshape(L, C, C)

    g1 = ln1_g[:, :, None]
    wqkv = np.concatenate([g1 * Wq_all, g1 * Wk_all, g1 * Wv_all], axis=2)
    bq = np.einsum("lc,lcd->ld", ln1_b, Wq_all)
    bk = np.einsum("lc,lcd->ld", ln1_b, Wk_all)
    bv = np.einsum("lc,lcd->ld", ln1_b, Wv_all)
    bo2 = bo + np.einsum("ld,ldc->lc", bv, Wo)       # v-bias folds through Wo
    w1f = ln2_g[:, :, None] * W1
    b1f = b1 + np.einsum("lc,lcd->ld", ln2_b, W1)
    wlmf = lnf_g[:, None] * Wlm
    blmf = blm + lnf_b @ Wlm

    bqkv = np.concatenate([bq, bk], axis=1)          # [L, 768]
    bqkv_cols = np.ascontiguousarray(
        bqkv.reshape(L, 6, 128).transpose(0, 2, 1)).astype(np.float32)
    b1_cols = np.ascontiguousarray(
        b1f.reshape(L, K12, 128).transpose(0, 2, 1)).astype(np.float32)
    brows = tobf(np.concatenate([bo2, b2], axis=1)[:, None, :])  # [L,1,2C]

    # multiplicative causal mask, key-major: cols = (key_block, q)
    p = np.arange(128)[:, None]
    q = np.arange(256)[None, :]
    m0 = (p <= q).astype(np.float32)          # keys 0..127
    m1 = (p + 128 <= q).astype(np.float32)    # keys 128..255
    m01 = tobf(np.concatenate([m0, m1], axis=1))    # [128, 512]

    x0 = tok_emb[idx] + pos_emb[None]                # [B,T,C] f32
    in_maps = []
    for c in range(N_CORES):
        x0c = x0[c * BPC:(c + 1) * BPC].reshape(NTOK, C)
        in_maps.append({
            "x0T": np.ascontiguousarray(x0c.T),
            "wqkv": tobf(wqkv),
            "bqkv": bqkv_cols,
            "wo": tobf(Wo),
            "w1": tobf(w1f),
            "b1": b1_cols,
            "w2": tobf(W2),
            "brows": brows,
            "wlm": tobf(wlmf),
            "blm": np.ascontiguousarray(blmf),
            "m01": m01,
        })
    return in_maps


def _run(inputs, trace=False):
    if "nc" not in _cache:
        _cache["nc"] = _build_nc()
    nc = _cache["nc"]
    in_maps = _host_prep(inputs)
    res = run_bass_kernel_spmd(nc, in_maps, core_ids=list(range(N_CORES)),
                               trace=trace)
    outs = []
    for c in range(N_CORES):
        outT = res.results[c]["outT"]                 # [V, NTOK]
        outs.append(outT.T.reshape(BPC, T, V))
    logits = np.concatenate(outs, axis=0).astype(np.float32)
    return logits, res


def kernel(**inputs) -> np.ndarray:
    logits, _ = _run(inputs, trace=False)
    return logits


# revision 24
# speedup vs baseline: 1.1834x; 1.0191x over previous
"""Bass/Trainium2 kernel for nn_CharLevelLanguageModel (6-layer char transformer).

v2 strategy: data-parallel over batch (64 -> 8 cores x 8). Per core, each layer
is emitted stage-major across the 4 batch-pair (512-token) tiles so that every
serial chain (LN row ops, softmax normalize) is covered by independent matmul
work from the other batch-pairs.

Key choices vs v1:
- bf16 matmul operands (weights + activations); residual stream and stats stay
  fp32 (f32r) in SBUF; PSUM accumulates fp32.
- Zero GpSimd instructions. Partition broadcasts are ones-column matmuls on the
  PE; elementwise work is split between DVE and ACT.
- Single ACT table set ("natural_log_exp_and_others"): rsqrt(v)=exp(-0.5*ln v),
  1/s = exp(-ln s). No table reloads after startup.
- LN affine params folded into adjacent weights on host; biases ride K=1
  bf16 matmul rows (brows x onesrow) or ACT bias columns.
- Causal mask is one multiplicative bf16 DVE op per (head, batch) unit.
"""

import os
import numpy as np
import ml_dtypes

import concourse.bass as bass
import concourse.mybir as mybir
import concourse.tile as tile
from concourse import bacc
from concourse.bass_utils import run_bass_kernel_spmd

B, T, C, H, L, V = 64, 256, 384, 6, 6, 65
HS = C // H          # 64
DFF = 4 * C          # 1536
N_CORES = 8
BPC = B // N_CORES   # 8 batches per core
NTOK = BPC * T       # 2048 tokens per core
NBP = 4              # batch-pair (512-token) tiles per core
KC = C // 128        # 3 feature chunks
K12 = DFF // 128     # 12 dff chunks
EPS = 1e-5
SCALE = HS ** -0.5

f32 = mybir.dt.float32
f32r = mybir.dt.float32r
bf16 = mybir.dt.bfloat16
i32 = mybir.dt.int32
AF = mybir.ActivationFunctionType
ALU = mybir.AluOpType

N_LAYERS = int(os.environ.get("KERNEL_LAYERS", str(L)))

_cache = {}


def _build_nc():
    nc = bacc.Bacc("TRN2", target_bir_lowering=False, debug=False,
                   num_devices=N_CORES)

    x0T_d = nc.dram_tensor("x0T", [C, NTOK], f32r, kind="ExternalInput").ap()
    wqkv_d = nc.dram_tensor("wqkv", [L, C, 3 * C], bf16, kind="ExternalInput").ap()
    bqkv_d = nc.dram_tensor("bqkv", [L, 128, 6], f32, kind="ExternalInput").ap()
    wo_d = nc.dram_tensor("wo", [L, C, C], bf16, kind="ExternalInput").ap()
    w1_d = nc.dram_tensor("w1", [L, C, DFF], bf16, kind="ExternalInput").ap()
    b1_d = nc.dram_tensor("b1", [L, 128, K12], f32, kind="ExternalInput").ap()
    w2_d = nc.dram_tensor("w2", [L, DFF, C], bf16, kind="ExternalInput").ap()
    bcols_d = nc.dram_tensor("bcols", [L, 128, 6], f32, kind="ExternalInput").ap()
    wlm_d = nc.dram_tensor("wlm", [C, V], bf16, kind="ExternalInput").ap()
    blm_d = nc.dram_tensor("blm", [V], f32, kind="ExternalInput").ap()
    m01_d = nc.dram_tensor("m01", [128, 512], bf16, kind="ExternalInput").ap()
    outT_d = nc.dram_tensor("outT", [V, NTOK], f32, kind="ExternalOutput").ap()

    with tile.TileContext(nc) as tc:
        _build_body(nc, tc, x0T_d, wqkv_d, bqkv_d, wo_d, w1_d, b1_d, w2_d,
                    bcols_d, wlm_d, blm_d, m01_d, outT_d)
    nc.compile()
    return nc


def _build_body(nc, tc, x0T_d, wqkv_d, bqkv_d, wo_d, w1_d, b1_d, w2_d,
                bcols_d, wlm_d, blm_d, m01_d, outT_d):
    import contextlib
    ctx = contextlib.ExitStack()
    p_const = ctx.enter_context(tc.tile_pool(name="consts", bufs=1))
    p_x = ctx.enter_context(tc.tile_pool(name="x", bufs=1))
    p_w = ctx.enter_context(tc.tile_pool(name="w", bufs=1))
    p_big = ctx.enter_context(tc.tile_pool(name="bigsb", bufs=1))   # xsq/xc
    p_xn = ctx.enter_context(tc.tile_pool(name="xn", bufs=1))
    p_rows = ctx.enter_context(tc.tile_pool(name="rows", bufs=1))
    p_qk = ctx.enter_context(tc.tile_pool(name="qk", bufs=1))
    p_v = ctx.enter_context(tc.tile_pool(name="v", bufs=1))
    p_e = ctx.enter_context(tc.tile_pool(name="e", bufs=1))
    p_attc = ctx.enter_context(tc.tile_pool(name="attc", bufs=1))
    p_a = ctx.enter_context(tc.tile_pool(name="a", bufs=1))
    p_out = ctx.enter_context(tc.tile_pool(name="out", bufs=2))
    ps_aux = ctx.enter_context(tc.tile_pool(name="ps_aux", bufs=2, space="PSUM"))
    ps_ap = ctx.enter_context(tc.tile_pool(name="ps_ap", bufs=2, space="PSUM"))
    ps_big = ctx.enter_context(tc.tile_pool(name="ps_big", bufs=4, space="PSUM"))

    # ---- constants ----
    stage = p_const.tile([128, 8], f32, tag="stage")
    onesC = p_const.tile([128, 2], f32r, tag="onesC")      # 1/C for mean matmuls
    nc.vector.memset(stage[:, 0:2], 1.0 / C)
    nc.vector.tensor_copy(onesC[:], stage[:, 0:2])
    stage_row = p_const.tile([1, 512], f32, tag="stage_row")
    nc.vector.memset(stage_row[:, 0:128], 1.0)
    onescol = p_const.tile([1, 128], f32r, tag="onescol")  # bcast lhsT
    nc.vector.tensor_copy(onescol[:], stage_row[:, 0:128])
    onescol34 = p_const.tile([34, 128], f32r, tag="onescol34")
    nc.vector.tensor_copy(onescol34[0:1, :], stage_row[:, 0:128])
    nc.vector.tensor_copy(onescol34[32:33, :], stage_row[:, 0:128])
    ones512r = p_const.tile([1, 512], f32r, tag="ones512r")
    nc.vector.memset(stage_row[:], 1.0)
    nc.vector.tensor_copy(ones512r[:], stage_row[:])
    epscol2 = p_const.tile([1, 2], f32r, tag="epscol2")    # eps rides sq-stats MM
    nc.vector.memset(stage[:, 2:4], EPS)
    nc.vector.tensor_copy(epscol2[:], stage[0:1, 2:4])
    m01 = p_const.tile([128, 512], bf16, tag="m01")
    nc.sync.dma_start(out=m01, in_=m01_d)
    blm_t = p_const.tile([V, 1], f32, tag="blm")
    nc.sync.dma_start(out=blm_t, in_=blm_d.rearrange("(v o) -> v o", o=1))
    wlm_t = [p_const.tile([128, V], bf16, tag=f"wlm{kc}", name=f"wlm{kc}")
             for kc in range(KC)]
    for kc in range(KC):
        nc.sync.dma_start(out=wlm_t[kc], in_=wlm_d[kc * 128:(kc + 1) * 128, :])

    # ---- residual stream: one [128, 3*512] f32 tile per batch-pair ----
    x_t = [p_x.tile([128, KC * 512], f32r, tag=f"x{nt}", name=f"x{nt}")
           for nt in range(NBP)]
    for nt in range(NBP):
        for kc in range(KC):
            nc.sync.dma_start(out=x_t[nt][:, kc * 512:(kc + 1) * 512],
                              in_=x0T_d[kc * 128:(kc + 1) * 128,
                                        nt * 512:nt * 512 + 512])

    # ---- V_ext buffers: ones column written once, V slices per layer ----
    vext = [[p_v.tile([128, 2 * H * (HS + 1)], bf16, tag=f"vext{nt}_{bi}",
                      name=f"vext{nt}_{bi}") for bi in range(2)]
            for nt in range(NBP)]
    for nt in range(NBP):
        for bi in range(2):
            vxr = vext[nt][bi].rearrange("p (j h e) -> p j h e", j=2, h=H)
            nc.vector.memset(vxr[:, :, :, HS:HS + 1], 1.0)

    weights = {}

    def load_wqkv(l):
        w = weights.setdefault(l, {})
        w["wqkv"] = [p_w.tile([128, 3 * C], bf16, tag=f"wqkv{kc}",
                              name=f"wqkv{kc}", bufs=2) for kc in range(KC)]
        for kc in range(KC):
            nc.sync.dma_start(out=w["wqkv"][kc],
                              in_=wqkv_d[l, kc * 128:(kc + 1) * 128, :])
        w["bqkv"] = p_w.tile([128, 6], f32, tag="bqkv", name="bqkv", bufs=2)
        nc.sync.dma_start(out=w["bqkv"], in_=bqkv_d[l])

    def load_rest(l):
        w = weights.setdefault(l, {})
        w["wo"] = [p_w.tile([128, C], bf16, tag=f"wo{kc}", name=f"wo{kc}",
                            bufs=2) for kc in range(KC)]
        for kc in range(KC):
            nc.sync.dma_start(out=w["wo"][kc],
                              in_=wo_d[l, kc * 128:(kc + 1) * 128, :])
        w["w1"] = [p_w.tile([128, DFF], bf16, tag=f"w1{kc}", name=f"w1{kc}",
                            bufs=2) for kc in range(KC)]
        for kc in range(KC):
            nc.sync.dma_start(out=w["w1"][kc],
                              in_=w1_d[l, kc * 128:(kc + 1) * 128, :])
        w["b1"] = p_w.tile([128, K12], f32, tag="b1", name="b1", bufs=2)
        nc.sync.dma_start(out=w["b1"], in_=b1_d[l])
        w["w2"] = [p_w.tile([128, C], bf16, tag=f"w2_{kc}", name=f"w2_{kc}",
                            bufs=2) for kc in range(K12)]
        for kc in range(K12):
            nc.sync.dma_start(out=w["w2"][kc],
                              in_=w2_d[l, kc * 128:(kc + 1) * 128, :])
        w["bcols"] = p_w.tile([128, 6], f32, tag="bcols", name="bcols",
                              bufs=2)
        nc.sync.dma_start(out=w["bcols"], in_=bcols_d[l])

    def stage_LN(nts):
        """Standardize x_t[nt] for each nt in `nts` (a pair of batch-pairs).

        Stats via ones-matmuls; the row chain (quake rsqrt seed + 2 Newton
        steps, all on DVE) is batched across the pair on a [34,512] tile
        (rows 0 and 32), since DVE small-op cost is per-instruction, not
        per-row. Per-token broadcasts are ones-column matmuls on the PE.
        Returns {nt: xn}.
        """
        P = 32 * (len(nts) - 1) + 2
        statps = {}
        for nt in nts:
            x = x_t[nt]
            xsq = p_big.tile([128, KC * 512], f32r, tag="xbig", name="xsq",
                             bufs=2)
            nc.vector.tensor_mul(xsq[:], x[:], x[:])
            mu_ps = ps_big.tile([2, 512], f32, tag="big", name="mu_ps")
            for kc in range(KC):
                nc.tensor.matmul(mu_ps[:], onesC[:],
                                 x[:, kc * 512:(kc + 1) * 512],
                                 start=(kc == 0), stop=(kc == KC - 1))
            sq_ps = ps_big.tile([2, 512], f32, tag="big", name="sq_ps")
            for kc in range(KC):
                nc.tensor.matmul(sq_ps[:], onesC[:],
                                 xsq[:, kc * 512:(kc + 1) * 512],
                                 start=(kc == 0), stop=False)
            nc.tensor.matmul(sq_ps[:], epscol2[:], ones512r[:], start=False,
                             stop=True)
            statps[nt] = (mu_ps, sq_ps)
        mu_sb = p_rows.tile([P, 512], f32r, tag="mu_sb", name="mu_sb", bufs=1)
        for i, nt in enumerate(nts):
            nc.scalar.copy(mu_sb[32 * i:32 * i + 1, :], statps[nt][0][0:1, :])
        musq = p_rows.tile([P, 512], f32r, tag="musq", name="musq", bufs=1)
        nc.scalar.activation(musq[:], mu_sb[:], AF.Square, bias=0.0, scale=1.0)
        # ve = E[x^2] + eps - mu^2   (junk in rows 2..31 is never read)
        ve = p_rows.tile([P, 512], f32, tag="ve", name="ve", bufs=2)
        for i, nt in enumerate(nts):
            nc.vector.tensor_tensor(out=ve[32 * i:32 * i + 1, :],
                                    in0=statps[nt][1][0:1, :],
                                    in1=musq[32 * i:32 * i + 1, :],
                                    op=ALU.subtract)
        ish = p_rows.tile([P, 512], i32, tag="ish", name="ish", bufs=1)
        nc.vector.tensor_scalar(out=ish[:], in0=ve[:].bitcast(i32), scalar1=1,
                                scalar2=None, op0=ALU.logical_shift_right)
        sdi = p_rows.tile([P, 512], i32, tag="sdi", name="sdi", bufs=1)
        nc.vector.tensor_scalar(out=sdi[:], in0=ish[:],
                                scalar1=float(0x5F3759DF), scalar2=-1.0,
                                op0=ALU.subtract, op1=ALU.mult)
        y0 = sdi[:].bitcast(f32)
        y1 = None
        for it_n in range(2):
            yv = y0 if it_n == 0 else y1[:]
            nt_a = p_rows.tile([P, 512], f32, tag="nt", name="nt", bufs=2)
            nc.vector.tensor_mul(nt_a[:], ve[:], yv)
            nt_b = p_rows.tile([P, 512], f32, tag="nt", name="nt", bufs=2)
            nc.vector.tensor_mul(nt_b[:], nt_a[:], yv)
            nt_c = p_rows.tile([P, 512], f32, tag="nc", name="nc", bufs=1)
            nc.vector.tensor_scalar(out=nt_c[:], in0=nt_b[:], scalar1=-0.5,
                                    scalar2=1.5, op0=ALU.mult, op1=ALU.add)
            dt_o = f32 if it_n == 0 else f32r
            tg = "y1" if it_n == 0 else "rs"
            y_n = p_rows.tile([P, 512], dt_o, tag=tg, name=tg, bufs=1)
            nc.vector.tensor_mul(y_n[:], nt_c[:], yv)
            y1 = y_n
        rs = y1
        xns = {}
        for i, nt in enumerate(nts):
            oc_i = onescol34[32 * i:32 * i + 1, :]
            mu_b = ps_aux.tile([128, 512], f32, tag="aux", name="mu_b")
            nc.tensor.matmul(mu_b[:], oc_i, mu_sb[32 * i:32 * i + 1, :],
                             start=True, stop=True)
            rs_b = ps_aux.tile([128, 512], f32, tag="aux", name="rs_b")
            nc.tensor.matmul(rs_b[:], oc_i, rs[32 * i:32 * i + 1, :],
                             start=True, stop=True)
            xn = p_xn.tile([128, KC * 512], bf16, tag=f"xn{nt}",
                           name=f"xn{nt}", bufs=1)
            xc = p_big.tile([128, KC * 512], f32r, tag="xbig", name="xc",
                            bufs=2)
            xv = x_t[nt][:].rearrange("p (k n) -> p k n", k=KC)
            xcv = xc[:].rearrange("p (k n) -> p k n", k=KC)
            xnv = xn[:].rearrange("p (k n) -> p k n", k=KC)
            mbv = mu_b[:].unsqueeze(1).to_broadcast((128, KC, 512))
            rbv = rs_b[:].unsqueeze(1).to_broadcast((128, KC, 512))
            nc.vector.tensor_tensor(out=xcv, in0=xv, in1=mbv,
                                    op=ALU.subtract)
            nc.vector.tensor_tensor(out=xnv, in0=xcv, in1=rbv, op=ALU.mult)
            xns[nt] = xn
        return xns

    state = {}

    def stage_A(l, nts):
        xns = stage_LN(nts)
        for nt in nts:
            state[nt] = {"xn": xns[nt]}

    def stage_B(l, nt):
        w = weights[l]
        xn = state[nt]["xn"]
        qk = []
        for oc in range(6):
            qp = ps_big.tile([128, 512], f32, tag="big", name="qp")
            for kc in range(KC):
                nc.tensor.matmul(qp[:], w["wqkv"][kc][:, oc * 128:oc * 128 + 128],
                                 xn[:, kc * 512:(kc + 1) * 512],
                                 start=(kc == 0), stop=(kc == KC - 1))
            qt = p_qk.tile([128, 512], bf16, tag=f"qk{oc}", name=f"qk{oc}",
                           bufs=2)
            nc.scalar.activation(qt[:], qp[:], AF.Identity,
                                 bias=w["bqkv"][:, oc:oc + 1], scale=1.0)
            qk.append(qt)
        for bi in range(2):
            vxr = vext[nt][bi].rearrange("p (j h e) -> p j h e", j=2, h=H)
            for j in range(2):
                vp = ps_big.tile([128, C], f32, tag="big", name="vp")
                tc0 = bi * 256 + j * 128
                for kc in range(KC):
                    nc.tensor.matmul(vp[:], xn[:, kc * 512 + tc0:kc * 512 + tc0 + 128],
                                     w["wqkv"][kc][:, 2 * C:3 * C],
                                     start=(kc == 0), stop=(kc == KC - 1))
                nc.scalar.copy(vxr[:, j, :, 0:HS],
                               vp[:].rearrange("p (h d) -> p h d", h=H))
        state[nt]["qk"] = qk

    def stage_CD(l, nt):
        """Wave-pipelined scores -> exp -> mask -> attV -> normalize."""
        st = state[nt]
        qk = st["qk"]
        attc = [p_attc.tile([128, 512], bf16, tag=f"attc{kc}",
                            name=f"attc{kc}", bufs=3) for kc in range(KC)]
        ap_t = {}
        e_ms = {}
        LAG = 2
        for u in range(12 + LAG):
            if u < 12:
                h, bi = divmod(u, 2)
                qch, kch = h // 2, 3 + h // 2
                qrow = (h % 2) * 64
                q0 = bi * 256
                sp = ps_big.tile([128, 512], f32, tag="big", name="sp")
                qs = qk[qch][qrow:qrow + 64, q0:q0 + 256]
                nc.tensor.matmul(sp[:, 0:256],
                                 qk[kch][qrow:qrow + 64, q0:q0 + 128],
                                 qs, start=True, stop=True)
                nc.tensor.matmul(sp[:, 256:512],
                                 qk[kch][qrow:qrow + 64, q0 + 128:q0 + 256],
                                 qs, start=True, stop=True)
                e_t = p_e.tile([128, 512], bf16, tag="e_t", name="e_t", bufs=3)
                nc.scalar.activation(e_t[:], sp[:], AF.Exp, bias=0.0,
                                     scale=SCALE)
                e_m = p_e.tile([128, 512], bf16, tag="e_m", name="e_m", bufs=4)
                nc.vector.tensor_mul(e_m[:], e_t[:], m01[:])
                e_ms[u] = e_m
            if u >= LAG:
                v = u - LAG
                h, bi = divmod(v, 2)
                qch = h // 2
                qrow = (h % 2) * 64
                q0 = bi * 256
                if bi == 0:
                    ap_t[h] = ps_ap.tile([HS + 1, 512], f32, tag="ap",
                                         name="ap_")
                ap_ = ap_t[h]
                e_m = e_ms.pop(v)
                vxr = vext[nt][bi].rearrange("p (j h e) -> p j h e", j=2, h=H)
                nc.tensor.matmul(ap_[:, q0:q0 + 256], vxr[:, 0, h, :],
                                 e_m[:, 0:256], start=True, stop=False)
                nc.tensor.matmul(ap_[:, q0:q0 + 256], vxr[:, 1, h, :],
                                 e_m[:, 256:512], start=False, stop=True)
                if bi == 1:
                    # 1/sum batched per head pair: copy both sum rows into a
                    # [34,512] tile (rows 0/32), one DVE reciprocal + one f32r
                    # rounding copy, then a 64-row broadcast matmul and one
                    # DVE multiply per head.
                    if h % 2 == 0:
                        srow = p_rows.tile([34, 512], f32, tag="srow",
                                           name="srow", bufs=1)
                        state[nt]["srow"] = srow
                        nc.scalar.copy(srow[0:1, :], ap_[HS:HS + 1, :])
                    else:
                        srow = state[nt]["srow"]
                        nc.scalar.copy(srow[32:33, :], ap_[HS:HS + 1, :])
                        rec = p_rows.tile([34, 512], f32, tag="rec",
                                          name="rec", bufs=1)
                        nc.vector.reciprocal_approx_fast(out=rec[:],
                                                         in_=srow[:])
                        rec_r = p_rows.tile([34, 512], f32r, tag="rec_r",
                                            name="rec_r", bufs=1)
                        nc.scalar.copy(rec_r[:], rec[:])
                        for hh in (h - 1, h):
                            rec_b = ps_aux.tile([64, 512], f32, tag="aux",
                                                name="rec_b")
                            bb = 32 * (hh % 2)
                            nc.tensor.matmul(rec_b[:],
                                             onescol34[bb:bb + 1, 0:64],
                                             rec_r[bb:bb + 1, :],
                                             start=True, stop=True)
                            rb_sb = p_rows.tile([64, 512], bf16, tag="rb_sb",
                                                name="rb_sb", bufs=2)
                            nc.scalar.copy(rb_sb[:], rec_b[:])
                            nc.vector.tensor_mul(
                                attc[qch][(hh % 2) * 64:(hh % 2) * 64 + 64, :],
                                ap_t[hh][0:HS, :], rb_sb[:])
        state[nt]["attc"] = attc
        del state[nt]["qk"], state[nt]["xn"], state[nt]["srow"]

    def stage_E(l, nt):
        w = weights[l]
        attc = state[nt]["attc"]
        for oc in range(KC):
            wp = ps_big.tile([128, 512], f32, tag="big", name="wp")
            for kc in range(KC):
                nc.tensor.matmul(wp[:], w["wo"][kc][:, oc * 128:oc * 128 + 128],
                                 attc[kc][:], start=(kc == 0),
                                 stop=(kc == KC - 1))
            sl = slice(oc * 512, (oc + 1) * 512)
            nc.vector.scalar_tensor_tensor(out=x_t[nt][:, sl], in0=wp[:],
                                           scalar=w["bcols"][:, oc:oc + 1],
                                           in1=x_t[nt][:, sl],
                                           op0=ALU.add, op1=ALU.add)
        del state[nt]["attc"]

    def stage_F(l, nts):
        xns = stage_LN(nts)
        for nt in nts:
            state[nt]["h2n"] = xns[nt]

    def stage_G(l, nt):
        w = weights[l]
        h2n = state[nt]["h2n"]
        a_t = []
        for kc12 in range(K12):
            fp1 = ps_big.tile([128, 512], f32, tag="big", name="fp1")
            for kc in range(KC):
                nc.tensor.matmul(fp1[:],
                                 w["w1"][kc][:, kc12 * 128:kc12 * 128 + 128],
                                 h2n[:, kc * 512:(kc + 1) * 512],
                                 start=(kc == 0), stop=(kc == KC - 1))
            a = p_a.tile([128, 512], bf16, tag=f"a{kc12}", name=f"a{kc12}",
                         bufs=1)
            nc.scalar.activation(a[:], fp1[:], AF.Relu,
                                 bias=w["b1"][:, kc12:kc12 + 1], scale=1.0)
            a_t.append(a)
        state[nt]["a"] = a_t
        del state[nt]["h2n"]

    def stage_H(l, nt):
        w = weights[l]
        a_t = state[nt]["a"]
        for oc in range(KC):
            fp2 = ps_big.tile([128, 512], f32, tag="big", name="fp2")
            for kc12 in range(K12):
                nc.tensor.matmul(fp2[:],
                                 w["w2"][kc12][:, oc * 128:oc * 128 + 128],
                                 a_t[kc12][:], start=(kc12 == 0),
                                 stop=(kc12 == K12 - 1))
            sl = slice(oc * 512, (oc + 1) * 512)
            nc.vector.scalar_tensor_tensor(out=x_t[nt][:, sl], in0=fp2[:],
                                           scalar=w["bcols"][:, 3 + oc:4 + oc],
                                           in1=x_t[nt][:, sl],
                                           op0=ALU.add, op1=ALU.add)
        del state[nt]

    def stage_HEAD(nts):
        xfs = stage_LN(nts)
        for nt in nts:
            xf = xfs[nt]
            lp = ps_big.tile([V, 512], f32, tag="big", name="lp")
            for kc in range(KC):
                nc.tensor.matmul(lp[:], wlm_t[kc][:],
                                 xf[:, kc * 512:(kc + 1) * 512],
                                 start=(kc == 0), stop=(kc == KC - 1))
            osb = p_out.tile([V, 512], f32, tag="osb", name="osb")
            nc.scalar.activation(osb[:], lp[:], AF.Identity, bias=blm_t[:],
                                 scale=1.0)
            nc.sync.dma_start(out=outT_d[:, nt * 512:nt * 512 + 512],
                              in_=osb[:])

    # ---- stage-major emission: 4 independent batch-pair streams per stage ----
    load_wqkv(0)
    load_rest(0)
    stage_A(0, [0, 1])
    stage_A(0, [2, 3])
    for l in range(N_LAYERS):
        for nt in range(NBP):
            stage_B(l, nt)
            stage_CD(l, nt)
        if l + 1 < N_LAYERS:
            load_wqkv(l + 1)
        stage_E(l, 0)
        stage_E(l, 1)
        stage_F(l, [0, 1])
        stage_E(l, 2)
        stage_E(l, 3)
        stage_F(l, [2, 3])
        if l + 1 < N_LAYERS:
            load_rest(l + 1)
        for pair in ([0, 1], [2, 3]):
            for nt in pair:
                stage_G(l, nt)
                stage_H(l, nt)
            if l + 1 < N_LAYERS:
                stage_A(l + 1, pair)
            else:
                stage_HEAD(pair)

    ctx.close()


def _host_prep(inputs):
    """Fold LN affine params into weights; build per-core input maps."""
    f = lambda k: np.asarray(inputs[k], dtype=np.float32)
    tobf = lambda a: np.ascontiguousarray(a.astype(ml_dtypes.bfloat16))
    idx = np.asarray(inputs["idx"]).astype(np.int64)
    tok_emb, pos_emb = f("tok_emb"), f("pos_emb")
    Wq, Wk, Wv, Wo = f("Wq"), f("Wk"), f("Wv"), f("Wo")
    bo, W1, b1, W2, b2 = f("bo"), f("W1"), f("b1"), f("W2"), f("b2")
    ln1_g, ln1_b = f("ln1_g"), f("ln1_b")
    ln2_g, ln2_b = f("ln2_g"), f("ln2_b")
    lnf_g, lnf_b = f("lnf_g"), f("lnf_b")
    Wlm, blm = f("Wlm"), f("blm")

    # [L,H,C,HS] -> [L,C,H*HS]
    Wq_all = np.transpose(Wq, (0, 2, 1, 3)).reshape(L, C, C)
    Wk_all = np.transpose(Wk, (0, 2, 1, 3)).reshape(L, C, C)
    Wv_all = np.transpose(Wv, (0, 2, 1, 3)).reshape(L, C, C)

    g1 = ln1_g[:, :, None]
    wqkv = np.concatenate([g1 * Wq_all, g1 * Wk_all, g1 * Wv_all], axis=2)
    bq = np.einsum("lc,lcd->ld", ln1_b, Wq_all)
    bk = np.einsum("lc,lcd->ld", ln1_b, Wk_all)
    bv = np.einsum("lc,lcd->ld", ln1_b, Wv_all)
    bo2 = bo + np.einsum("ld,ldc->lc", bv, Wo)       # v-bias folds through Wo
    w1f = ln2_g[:, :, None] * W1
    b1f = b1 + np.einsum("lc,lcd->ld", ln2_b, W1)
    wlmf = lnf_g[:, None] * Wlm
    blmf = blm + lnf_b @ Wlm

    bqkv = np.concatenate([bq, bk], axis=1)          # [L, 768]
    bqkv_cols = np.ascontiguousarray(
        bqkv.reshape(L, 6, 128).transpose(0, 2, 1)).astype(np.float32)
    b1_cols = np.ascontiguousarray(
        b1f.reshape(L, K12, 128).transpose(0, 2, 1)).astype(np.float32)
    bcols = np.ascontiguousarray(np.concatenate(
        [bo2.reshape(L, KC, 128), b2.reshape(L, KC, 128)],
        axis=1).transpose(0, 2, 1)).astype(np.float32)  # [L,128,6]

    # multiplicative causal mask, key-major: cols = (key_block, q)
    p = np.arange(128)[:, None]
    q = np.arange(256)[None, :]
    m0 = (p <= q).astype(np.float32)          # keys 0..127
    m1 = (p + 128 <= q).astype(np.float32)    # keys 128..255
    m01 = tobf(np.concatenate([m0, m1], axis=1))    # [128, 512]

    x0 = tok_emb[idx] + pos_emb[None]                # [B,T,C] f32
    in_maps = []
    for c in range(N_CORES):
        x0c = x0[c * BPC:(c + 1) * BPC].reshape(NTOK, C)
        in_maps.append({
            "x0T": np.ascontiguousarray(x0c.T),
            "wqkv": tobf(wqkv),
            "bqkv": bqkv_cols,
            "wo": tobf(Wo),
            "w1": tobf(w1f),
            "b1": b1_cols,
            "w2": tobf(W2),
            "bcols": bcols,
            "wlm": tobf(wlmf),
            "blm": np.ascontiguousarray(blmf),
            "m01": m01,
        })
    return in_maps


def _run(inputs, trace=False):
    if "nc" not in _cache:
        _cache["nc"] = _build_nc()
    nc = _cache["nc"]
    in_maps = _host_prep(inputs)
    res = run_bass_kernel_spmd(nc, in_maps, core_ids=list(range(N_CORES)),
                               trace=trace)
    outs = []
    for c in range(N_CORES):
        outT = res.results[c]["outT"]                 # [V, NTOK]
        outs.append(outT.T.reshape(BPC, T, V))
    logits = np.concatenate(outs, axis=0).astype(np.float32)
    return logits, res


def kernel(**inputs) -> np.ndarray:
    logits, _ = _run(inputs, trace=False)
    return logits
